# revision 1
# baseline (speedup 1.0000x reference)
"""Distributed GQA attention kernel for 8 TRN2 NeuronCores.

Strategy: tensor-parallel over heads, zero collectives.
Each core d holds 4 query heads + 1 kv head (GQA group d). It computes
q/k/v projections (transposed layouts), RoPE, causal attention, and a
partial o_proj (its heads' contribution to every output element). The
host sums the 8 partial outputs (the "unshard" step).

Precision: bf16 matmuls with f32 PSUM accumulation; softmax in f32
without max-subtraction (scores are ~N(0,1); exp cannot overflow).
RoPE uses a head-dim permutation (even indices first) applied to
wq/wk/wo rows on the host, turning the rotate-pair swap into a
contiguous 64-partition shift on device.
"""
import sys

sys.path.insert(0, '/opt/trn_rl_repo')

import numpy as np
import ml_dtypes

B, T, C = 2, 2048, 4096
H, KVH, HD = 32, 8, 128
NCORES = 8
N = B * T            # 4096 tokens (batches concatenated)
HL = H // NCORES     # 4 local q heads
TB = 256             # token block (q-tile)
NTB = N // TB        # 16
KB = 128             # key block
NCH = C // 128       # 32 contraction chunks
SCALE = float(1.0 / np.sqrt(HD))
PERM = np.concatenate([np.arange(0, 128, 2), np.arange(1, 128, 2)])

BF16 = ml_dtypes.bfloat16


def _build(dbg=False):
    import concourse.mybir as mybir
    import concourse.tile as tile
    from concourse import bacc

    dt = mybir.dt
    nc = bacc.Bacc("TRN2", target_bir_lowering=False, debug=False)

    xT_d = nc.declare_dram_parameter("xT", [C, N], dt.bfloat16, isOutput=False)
    wqT_d = nc.declare_dram_parameter("wqT", [C, HL * HD], dt.bfloat16, isOutput=False)
    wkT_d = nc.declare_dram_parameter("wkT", [C, HD], dt.bfloat16, isOutput=False)
    wvT_d = nc.declare_dram_parameter("wvT", [C, HD], dt.bfloat16, isOutput=False)
    woT_d = nc.declare_dram_parameter("woT", [HL * HD, C], dt.bfloat16, isOutput=False)
    cosb_d = nc.declare_dram_parameter("cosb", [128, N], dt.float32, isOutput=False)
    sinb_d = nc.declare_dram_parameter("sinb", [128, N], dt.float32, isOutput=False)
    mask_d = nc.declare_dram_parameter("mask", [128, 2 * TB], dt.bfloat16, isOutput=False)
    out_d = nc.declare_dram_parameter("out", [N, C], dt.bfloat16, isOutput=True)
    if dbg:
        kc_d = nc.declare_dram_parameter("kc", [128, N], dt.bfloat16, isOutput=True)
        vc_d = nc.declare_dram_parameter("vc", [128, N], dt.bfloat16, isOutput=True)
        q0_d = nc.declare_dram_parameter("q0", [128, N], dt.bfloat16, isOutput=True)
        a0_d = nc.declare_dram_parameter("a0", [128, N], dt.bfloat16, isOutput=True)

    with tile.TileContext(nc) as tc:
        with (
            tc.tile_pool(name="wts", bufs=1) as wts,
            tc.tile_pool(name="cache", bufs=1) as cache,
            tc.tile_pool(name="xin", bufs=6) as xin,
            tc.tile_pool(name="qk", bufs=10) as qkp,
            tc.tile_pool(name="rope", bufs=6) as ropep,
            tc.tile_pool(name="pt", bufs=4) as ptp,
            tc.tile_pool(name="att", bufs=10) as attp,
            tc.tile_pool(name="dn", bufs=4) as dnp,
            tc.tile_pool(name="oev", bufs=4) as oevp,
            tc.tile_pool(name="acc", bufs=3, space="PSUM") as accp,
            tc.tile_pool(name="sps", bufs=2, space="PSUM") as spsp,
            tc.tile_pool(name="atd", bufs=2, space="PSUM") as atdp,
            tc.tile_pool(name="ops", bufs=1, space="PSUM") as opsp,
        ):
            # ---------------- resident weights / constants ----------------
            wq_s = wts.tile([128, NCH * HL * 128], dt.bfloat16)   # (c,h) -> col (c*HL+h)*128
            wk_s = wts.tile([128, NCH * 128], dt.bfloat16)
            wv_s = wts.tile([128, NCH * 128], dt.bfloat16)
            wo_s = wts.tile([128, HL * C], dt.bfloat16)           # (h,ct) -> col h*C+ct*512
            cos_s = wts.tile([128, N], dt.float32)
            sin_s = wts.tile([128, N], dt.float32)
            mask_s = wts.tile([128, 2 * TB], dt.bfloat16)
            ones_s = wts.tile([128, 128], dt.bfloat16)

            nc.any.memset(ones_s[:, :], 1.0)
            # split big weight loads into 8 pieces for DMA-queue parallelism
            wq_v = wq_s[:, :].rearrange("p (c m) -> p c m", c=NCH)
            wqT_v = wqT_d[:, :].rearrange("(c p) m -> p c m", p=128)
            wo_v = wo_s[:, :].rearrange("p (h m) -> p h m", h=HL)
            woT_v = woT_d[:, :].rearrange("(h p) m -> p h m", p=128)
            for i in range(8):
                cs = slice(i * (NCH // 8), (i + 1) * (NCH // 8))
                nc.sync.dma_start(wq_v[:, cs], wqT_v[:, cs])
            for i in range(HL):
                nc.sync.dma_start(wo_v[:, i], woT_v[:, i])
            wk_v = wk_s[:, :].rearrange("p (c m) -> p c m", c=NCH)
            wkT_v = wkT_d[:, :].rearrange("(c p) m -> p c m", p=128)
            wv_v = wv_s[:, :].rearrange("p (c m) -> p c m", c=NCH)
            wvT_v = wvT_d[:, :].rearrange("(c p) m -> p c m", p=128)
            for i in range(4):
                cs = slice(i * (NCH // 4), (i + 1) * (NCH // 4))
                nc.sync.dma_start(wk_v[:, cs], wkT_v[:, cs])
                nc.sync.dma_start(wv_v[:, cs], wvT_v[:, cs])
            for i in range(4):
                ns = slice(i * (N // 4), (i + 1) * (N // 4))
                nc.sync.dma_start(cos_s[:, ns], cosb_d[:, ns])
                nc.sync.dma_start(sin_s[:, ns], sinb_d[:, ns])
            nc.sync.dma_start(mask_s[:, :], mask_d[:, :])

            kcache = cache.tile([128, N], dt.bfloat16)   # [hd, tok]
            vcache = cache.tile([128, N], dt.bfloat16)   # [tok%128, blk*128+hd]

            for tb in range(NTB):
                b = tb // 8
                nsl = slice(tb * TB, (tb + 1) * TB)
                # ================= A: projections for this token block ====
                t0 = accp.tile([128, 512], dt.float32, tag="acc")  # q0|q1
                t1 = accp.tile([128, 512], dt.float32, tag="acc")  # q2|q3
                t2 = accp.tile([128, 512], dt.float32, tag="acc")  # k|v0|v1
                qps = [t0[:, 0:256], t0[:, 256:512], t1[:, 0:256], t1[:, 256:512]]
                kps = t2[:, 0:256]
                vps = [t2[:, 256:384], t2[:, 384:512]]
                for c in range(NCH):
                    xc = xin.tile([128, TB], dt.bfloat16, tag="xc")
                    nc.sync.dma_start(xc[:, :], xT_d[c * 128:(c + 1) * 128, nsl])
                    st = c == 0
                    sp = c == NCH - 1
                    # start=True clears has_written for the WHOLE bank, so only
                    # the first matmul touching each bank may set it; sibling
                    # slices overwrite via cleared has_written on their first
                    # write (PE executes in program order).
                    for h in range(HL):
                        nc.tensor.matmul(
                            qps[h], wq_s[:, (c * HL + h) * 128:(c * HL + h + 1) * 128],
                            xc[:, :], start=st and h % 2 == 0, stop=sp)
                    nc.tensor.matmul(
                        kps, wk_s[:, c * 128:(c + 1) * 128], xc[:, :],
                        start=st, stop=sp)
                    for ti in range(2):
                        nc.tensor.matmul(
                            vps[ti], xc[:, ti * 128:(ti + 1) * 128],
                            wv_s[:, c * 128:(c + 1) * 128], start=False, stop=sp)

                # ---- RoPE + evacuate q (4 heads) and k; copy v to cache ----
                q_sb = []
                for h in range(HL + 1):  # h==HL is k
                    src = kps if h == HL else qps[h]
                    m1 = ropep.tile([128, TB], dt.float32, tag="m1")
                    nc.vector.tensor_mul(m1[:, :], src, cos_s[:, nsl])
                    u = ropep.tile([128, TB], dt.float32, tag="u")
                    nc.vector.tensor_mul(u[:, :], src, sin_s[:, nsl])
                    sw = ropep.tile([128, TB], dt.float32, tag="sw")
                    nc.sync.dma_start(sw[0:64, :], u[64:128, :])
                    nc.sync.dma_start(sw[64:128, :], u[0:64, :])
                    if h == HL:
                        nc.vector.tensor_add(kcache[:, nsl], m1[:, :], sw[:, :])
                    else:
                        qh = qkp.tile([128, TB], dt.bfloat16, tag="qh")
                        nc.vector.tensor_add(qh[:, :], m1[:, :], sw[:, :])
                        q_sb.append(qh)
                for ti in range(2):
                    kbg = tb * 2 + ti
                    nc.vector.tensor_copy(
                        vcache[:, kbg * 128:(kbg + 1) * 128], vps[ti])

                # ================= B: attention for this token block =======
                attn_sb = []
                nkb = 2 * (tb % 8) + 2
                for h in range(HL):
                    atd = atdp.tile([128, 512], dt.float32, tag="atd")
                    at = atd[:, 0:256]
                    den = atd[:, 256:512]
                    for kbl in range(nkb):
                        kbg = b * 16 + kbl
                        ksl = slice(kbg * 128, (kbg + 1) * 128)
                        sT = spsp.tile([128, TB], dt.float32, tag="sT")
                        nc.tensor.matmul(sT[:, :], kcache[:, ksl], q_sb[h][:, :],
                                         start=True, stop=True)
                        pT = ptp.tile([128, TB], dt.bfloat16, tag="pT")
                        nc.scalar.activation(pT[:, :], sT[:, :],
                                             mybir.ActivationFunctionType.Exp,
                                             scale=SCALE)
                        sub = kbl - 2 * (tb % 8)
                        if sub >= 0:
                            nc.vector.tensor_mul(
                                pT[:, :], pT[:, :],
                                mask_s[:, sub * TB:(sub + 1) * TB])
                        st = kbl == 0
                        sp = kbl == nkb - 1
                        nc.tensor.matmul(at, vcache[:, ksl], pT[:, :],
                                         start=st, stop=sp)
                        nc.tensor.matmul(den, ones_s[:, :], pT[:, :],
                                         start=False, stop=sp)
                    denb = dnp.tile([128, TB], dt.float32, tag="denb")
                    nc.vector.reciprocal(denb[:, :], den)
                    ah = attp.tile([128, TB], dt.bfloat16, tag="ah")
                    nc.vector.tensor_mul(ah[:, :], at, denb[:, :])
                    attn_sb.append(ah)
                    if dbg and h == 0:
                        nc.sync.dma_start(a0_d[:, nsl], ah[:, :])
                        nc.sync.dma_start(q0_d[:, nsl], q_sb[0][:, :])

                # ================= C: partial o_proj =======================
                for ti in range(2):
                    r0 = tb * TB + ti * 128
                    for ct in range(C // 512):
                        ops = opsp.tile([128, 512], dt.float32, tag="ops")
                        for h in range(HL):
                            nc.tensor.matmul(
                                ops[:, :],
                                attn_sb[h][:, ti * 128:(ti + 1) * 128],
                                wo_s[:, h * C + ct * 512:h * C + (ct + 1) * 512],
                                start=(h == 0), stop=(h == HL - 1))
                        oev = oevp.tile([128, 512], dt.bfloat16, tag="oev")
                        nc.vector.tensor_copy(oev[:, :], ops[:, :])
                        nc.sync.dma_start(
                            out_d[r0:r0 + 128, ct * 512:(ct + 1) * 512], oev[:, :])
            if dbg:
                nc.sync.dma_start(kc_d[:, :], kcache[:, :])
                nc.sync.dma_start(vc_d[:, :], vcache[:, :])
    nc.finalize()
    return nc


def _prep_shared(x, freqs_cis):
    xf = np.asarray(x, np.float32).reshape(N, C)
    xT = np.ascontiguousarray(xf.T).astype(BF16)
    fc = np.asarray(freqs_cis, np.float32)
    cos = fc[:, :, 0]
    sin = fc[:, :, 1]
    cosb = np.ascontiguousarray(np.tile(np.concatenate([cos.T, cos.T], 0), (1, B)), dtype=np.float32)
    # pre-swapped sin: device computes u = q*sinb then rotates u by 64
    # partitions, giving swap64(q)*(-sin | +sin) as RoPE needs.
    sinb = np.ascontiguousarray(np.tile(np.concatenate([sin.T, -sin.T], 0), (1, B)), dtype=np.float32)
    j = np.arange(KB)[:, None]
    qq = np.arange(TB)[None, :]
    mask = np.concatenate(
        [(sub * KB + j <= qq).astype(np.float32) for sub in (0, 1)], axis=1
    ).astype(BF16)
    return xT, cosb, sinb, mask


def _prep_core(d, wq_p, wk_p, wv_f, wo_f):
    qsl = slice(d * HL * HD, (d + 1) * HL * HD)
    ksl = slice(d * HD, (d + 1) * HD)
    wqT = np.ascontiguousarray(wq_p[qsl].T).astype(BF16)
    wkT = np.ascontiguousarray(wk_p[ksl].T).astype(BF16)
    wvT = np.ascontiguousarray(wv_f[ksl].T).astype(BF16)
    woT = np.ascontiguousarray(wo_f[:, qsl].T).astype(BF16)
    return wqT, wkT, wvT, woT


_NC_CACHE = []


def kernel(x, freqs_cis, wq, wk, wv, wo):
    from concourse import bass_utils

    if not _NC_CACHE:
        _NC_CACHE.append(_build())
    nc = _NC_CACHE[0]

    xT, cosb, sinb, mask = _prep_shared(x, freqs_cis)
    wq_p = np.asarray(wq, np.float32).reshape(H, HD, C)[:, PERM, :].reshape(H * HD, C)
    wk_p = np.asarray(wk, np.float32).reshape(KVH, HD, C)[:, PERM, :].reshape(KVH * HD, C)
    wv_f = np.asarray(wv, np.float32)
    wo_f = np.asarray(wo, np.float32)

    in_maps = []
    for d in range(NCORES):
        wqT, wkT, wvT, woT = _prep_core(d, wq_p, wk_p, wv_f, wo_f)
        in_maps.append({
            "xT": xT, "wqT": wqT, "wkT": wkT, "wvT": wvT, "woT": woT,
            "cosb": cosb, "sinb": sinb, "mask": mask,
        })
    res = bass_utils.run_bass_kernel_spmd(nc, in_maps, core_ids=list(range(NCORES)))
    acc = np.zeros((N, C), np.float32)
    for r in res.results:
        acc += np.asarray(r["out"], np.float32)
    return acc.reshape(B, T, C)



# revision 4
# speedup vs baseline: 1.0392x; 1.0392x over previous
"""Distributed GQA attention kernel for 8 TRN2 NeuronCores.

Strategy: tensor-parallel over heads, zero collectives.
Each core d holds 4 query heads + 1 kv head (GQA group d). It computes
q/k/v projections (transposed layouts), RoPE, causal attention, and a
partial o_proj (its heads' contribution to every output element). The
host sums the 8 partial outputs (the "unshard" step).

v2 layout decisions (all aimed at keeping the PE busy):
- Attention runs at 128-query granularity with all 4 local heads packed
  side by side, so score/AV/den matmuls stream 512 columns each.
- RoPE's rotate-half is a DVE stream_shuffle (32-lane group swap); the
  head-dim permutation is chosen so each pair's partner sits 16
  partitions away inside the same 32-partition quadrant.
- o_proj PSUM is double-buffered and its evacuations alternate between
  the Vector and Scalar engines.
- x is loaded 4 contraction-chunks per DMA; the o_proj partial output is
  written with one DMA per 128-token row block.
"""
import sys

sys.path.insert(0, '/opt/trn_rl_repo')

import numpy as np
import ml_dtypes

B, T, C = 2, 2048, 4096
H, KVH, HD = 32, 8, 128
NCORES = 8
N = B * T            # 4096 tokens (batches concatenated)
HL = H // NCORES     # 4 local q heads
TB = 256             # token block for projections
NTB = N // TB        # 16
QB = 128             # query block for attention
KB = 128             # key block
NCH = C // 128       # 32 contraction chunks
SCALE = float(1.0 / np.sqrt(HD))

# Head-dim permutation: pair i=(2i,2i+1) lives in quadrant i//16 at
# offsets i%16 (the "a" half) and 16+i%16 (the "b" half), so rotate-half
# becomes a 16<->16 swap inside each 32-partition stream_shuffle group.
PERM = np.empty(128, np.int64)
for _p in range(128):
    _qd, _r = _p // 32, _p % 32
    _i = _qd * 16 + (_r % 16)
    PERM[_p] = 2 * _i + (1 if _r >= 16 else 0)
IMAP = (np.arange(128) // 32) * 16 + (np.arange(128) % 32) % 16
SSIGN = np.where((np.arange(128) % 32) < 16, 1.0, -1.0).astype(np.float32)
SHUF = [(i + 16) % 32 for i in range(32)]

BF16 = ml_dtypes.bfloat16


def _build(dbg=False):
    import concourse.mybir as mybir
    import concourse.tile as tile
    from concourse import bacc

    dt = mybir.dt
    nc = bacc.Bacc("TRN2", target_bir_lowering=False, debug=False)

    xT_d = nc.declare_dram_parameter("xT", [C, N], dt.bfloat16, isOutput=False)
    wqT_d = nc.declare_dram_parameter("wqT", [C, HL * HD], dt.bfloat16, isOutput=False)
    wkT_d = nc.declare_dram_parameter("wkT", [C, HD], dt.bfloat16, isOutput=False)
    wvT_d = nc.declare_dram_parameter("wvT", [C, HD], dt.bfloat16, isOutput=False)
    woT_d = nc.declare_dram_parameter("woT", [HL * HD, C], dt.bfloat16, isOutput=False)
    cosb_d = nc.declare_dram_parameter("cosb", [128, N], dt.float32, isOutput=False)
    sinb_d = nc.declare_dram_parameter("sinb", [128, N], dt.float32, isOutput=False)
    mask_d = nc.declare_dram_parameter("mask", [128, HL * QB], dt.bfloat16, isOutput=False)
    out_d = nc.declare_dram_parameter("out", [N, C], dt.bfloat16, isOutput=True)

    with tile.TileContext(nc) as tc:
        with (
            tc.tile_pool(name="wts", bufs=1) as wts,
            tc.tile_pool(name="cache", bufs=1) as cache,
            tc.tile_pool(name="xin", bufs=12) as xin,
            tc.tile_pool(name="qk", bufs=2) as qkp,
            tc.tile_pool(name="rope", bufs=6) as ropep,
            tc.tile_pool(name="pt", bufs=6) as ptp,
            tc.tile_pool(name="att", bufs=2) as attp,
            tc.tile_pool(name="dn", bufs=2) as dnp,
            tc.tile_pool(name="oev", bufs=2) as oevp,
            tc.tile_pool(name="acc", bufs=2, space="PSUM") as accp,
            tc.tile_pool(name="sps", bufs=2, space="PSUM") as spsp,
            tc.tile_pool(name="avp", bufs=1, space="PSUM") as avpp,
            tc.tile_pool(name="dnp", bufs=1, space="PSUM") as dppp,
            tc.tile_pool(name="ops", bufs=2, space="PSUM") as opsp,
        ):
            # ---------------- resident weights / constants ----------------
            wq_s = wts.tile([128, NCH * HL * 128], dt.bfloat16)   # (c,h) -> col (c*HL+h)*128
            wk_s = wts.tile([128, NCH * 128], dt.bfloat16)
            wv_s = wts.tile([128, NCH * 128], dt.bfloat16)
            wo_s = wts.tile([128, HL * C], dt.bfloat16)           # (h,ct) -> col h*C+ct*512
            cos_s = wts.tile([128, N], dt.float32)
            sin_s = wts.tile([128, N], dt.float32)
            mask_s = wts.tile([128, HL * QB], dt.bfloat16)
            ones_s = wts.tile([128, 128], dt.bfloat16)

            nc.any.memset(ones_s[:, :], 1.0)
            # split big weight loads into pieces for DMA-queue parallelism
            wq_v = wq_s[:, :].rearrange("p (c m) -> p c m", c=NCH)
            wqT_v = wqT_d[:, :].rearrange("(c p) m -> p c m", p=128)
            wo_v = wo_s[:, :].rearrange("p (h m) -> p h m", h=HL)
            woT_v = woT_d[:, :].rearrange("(h p) m -> p h m", p=128)
            for i in range(8):
                cs = slice(i * (NCH // 8), (i + 1) * (NCH // 8))
                nc.sync.dma_start(wq_v[:, cs], wqT_v[:, cs])
            for i in range(HL):
                nc.sync.dma_start(wo_v[:, i], woT_v[:, i])
            wk_v = wk_s[:, :].rearrange("p (c m) -> p c m", c=NCH)
            wkT_v = wkT_d[:, :].rearrange("(c p) m -> p c m", p=128)
            wv_v = wv_s[:, :].rearrange("p (c m) -> p c m", c=NCH)
            wvT_v = wvT_d[:, :].rearrange("(c p) m -> p c m", p=128)
            for i in range(4):
                cs = slice(i * (NCH // 4), (i + 1) * (NCH // 4))
                nc.sync.dma_start(wk_v[:, cs], wkT_v[:, cs])
                nc.sync.dma_start(wv_v[:, cs], wvT_v[:, cs])
            for i in range(4):
                ns = slice(i * (N // 4), (i + 1) * (N // 4))
                nc.sync.dma_start(cos_s[:, ns], cosb_d[:, ns])
                nc.sync.dma_start(sin_s[:, ns], sinb_d[:, ns])
            nc.sync.dma_start(mask_s[:, :], mask_d[:, :])

            kcache = cache.tile([128, N], dt.bfloat16)   # [hd, tok]
            vcache = cache.tile([128, N], dt.bfloat16)   # [tok%128, blk*128+hd]

            def rope(dst, src, nsl):
                # dst = src*cos + shuffle16(src*sin'); all [128, TB]
                m1 = ropep.tile([128, TB], dt.float32, tag="m1")
                nc.vector.tensor_mul(m1[:, :], src, cos_s[:, nsl])
                u = ropep.tile([128, TB], dt.float32, tag="u")
                nc.vector.tensor_mul(u[:, :], src, sin_s[:, nsl])
                sw = ropep.tile([128, TB], dt.float32, tag="sw")
                nc.vector.stream_shuffle(sw[:, :], u[:, :], SHUF)
                nc.vector.tensor_add(dst, m1[:, :], sw[:, :])

            for tb in range(NTB):
                b = tb // 8
                nsl = slice(tb * TB, (tb + 1) * TB)
                # ================= A: projections for this token block ====
                # pass 1: q0|q1 and k|v0|v1 (2 banks); pass 2: q2|q3.
                xcs = []
                for ci in range(8):
                    xc = xin.tile([128, 4 * TB], dt.bfloat16, tag="xc")
                    xc_v = xc[:, :].rearrange("p (c m) -> p c m", c=4)
                    nc.sync.dma_start(
                        xc_v[:, :, :],
                        xT_d[:, nsl].rearrange("(c p) m -> p c m", p=128)[
                            :, ci * 4:(ci + 1) * 4])
                    xcs.append(xc_v)
                # q_sb layout: [hd, (h, qh, 128)] so the attention rhs for
                # query-half qh is the strided view [:, :, qh, :] (512 wide)
                q_sb = qkp.tile([128, HL * TB], dt.bfloat16, tag="qsb")
                q_qv = q_sb[:, :].rearrange("p (h q) -> p h q", h=HL)

                t0 = accp.tile([128, 512], dt.float32, tag="acc")  # q0|q1
                t2 = accp.tile([128, 512], dt.float32, tag="acc")  # k|v0|v1
                for c in range(NCH):
                    xc = xcs[c // 4][:, c % 4, :]
                    st = c == 0
                    sp = c == NCH - 1
                    for h in range(2):
                        nc.tensor.matmul(
                            t0[:, h * 256:(h + 1) * 256],
                            wq_s[:, (c * HL + h) * 128:(c * HL + h + 1) * 128],
                            xc, start=st and h == 0, stop=sp)
                    nc.tensor.matmul(
                        t2[:, 0:256],
                        wk_s[:, c * 128:(c + 1) * 128], xc,
                        start=st, stop=sp)
                    for ti in range(2):
                        nc.tensor.matmul(
                            t2[:, 256 + ti * 128:256 + (ti + 1) * 128],
                            xc[:, ti * 128:(ti + 1) * 128],
                            wv_s[:, c * 128:(c + 1) * 128], start=False, stop=sp)
                rope(q_qv[:, 0, :], t0[:, 0:256], nsl)
                rope(q_qv[:, 1, :], t0[:, 256:512], nsl)
                rope(kcache[:, nsl], t2[:, 0:256], nsl)
                for ti in range(2):
                    kbg = tb * 2 + ti
                    nc.vector.tensor_copy(
                        vcache[:, kbg * 128:(kbg + 1) * 128],
                        t2[:, 256 + ti * 128:256 + (ti + 1) * 128])
                t1 = accp.tile([128, 512], dt.float32, tag="acc")  # q2|q3
                for c in range(NCH):
                    xc = xcs[c // 4][:, c % 4, :]
                    sp = c == NCH - 1
                    for h in range(2):
                        nc.tensor.matmul(
                            t1[:, h * 256:(h + 1) * 256],
                            wq_s[:, (c * HL + h + 2) * 128:(c * HL + h + 3) * 128],
                            xc, start=c == 0 and h == 0, stop=sp)
                rope(q_qv[:, 2, :], t1[:, 0:256], nsl)
                rope(q_qv[:, 3, :], t1[:, 256:512], nsl)

                # ============ B+C: attention + o_proj per query block =====
                for qh in range(2):
                    qbl = (tb % 8) * 2 + qh        # in-batch 128-query block
                    nkb = qbl + 1
                    qrhs = q_qv[:, :, qh * 128:(qh + 1) * 128]
                    at4 = avpp.tile([128, 512], dt.float32, tag="at4")
                    den = dppp.tile([128, 512], dt.float32, tag="den")
                    for kbl in range(nkb):
                        kbg = b * 16 + kbl
                        ksl = slice(kbg * 128, (kbg + 1) * 128)
                        sT = spsp.tile([128, 512], dt.float32, tag="sT")
                        nc.tensor.matmul(sT[:, :], kcache[:, ksl], qrhs,
                                         start=True, stop=True)
                        pT = ptp.tile([128, 512], dt.bfloat16, tag="pT")
                        nc.scalar.activation(pT[:, :], sT[:, :],
                                             mybir.ActivationFunctionType.Exp,
                                             scale=SCALE)
                        if kbl == nkb - 1:
                            nc.vector.tensor_mul(pT[:, :], pT[:, :], mask_s[:, :])
                        st = kbl == 0
                        sp = kbl == nkb - 1
                        nc.tensor.matmul(at4[:, :], vcache[:, ksl], pT[:, :],
                                         start=st, stop=sp)
                        nc.tensor.matmul(den[:, :], ones_s[:, :], pT[:, :],
                                         start=st, stop=sp)
                    denb = dnp.tile([128, 512], dt.float32, tag="denb")
                    nc.vector.reciprocal(denb[:, :], den[:, :])
                    attn = attp.tile([128, 512], dt.bfloat16, tag="attn")
                    nc.vector.tensor_mul(attn[:, :], at4[:, :], denb[:, :])

                    # -------- C: partial o_proj for these 128 tokens ------
                    r0 = tb * TB + qh * 128
                    oev = oevp.tile([128, C], dt.bfloat16, tag="oev")
                    for ct in range(C // 512):
                        ops = opsp.tile([128, 512], dt.float32, tag="ops")
                        for h in range(HL):
                            nc.tensor.matmul(
                                ops[:, :],
                                attn[:, h * 128:(h + 1) * 128],
                                wo_s[:, h * C + ct * 512:h * C + (ct + 1) * 512],
                                start=(h == 0), stop=(h == HL - 1))
                        osl = slice(ct * 512, (ct + 1) * 512)
                        if ct % 2 == 0:
                            nc.vector.tensor_copy(oev[:, osl], ops[:, :])
                        else:
                            nc.scalar.activation(
                                oev[:, osl], ops[:, :],
                                mybir.ActivationFunctionType.Copy)
                    nc.sync.dma_start(out_d[r0:r0 + 128, :], oev[:, :])
    nc.finalize()
    return nc


def _prep_shared(x, freqs_cis):
    xf = np.asarray(x, np.float32).reshape(N, C)
    xT = np.ascontiguousarray(xf.T).astype(BF16)
    fc = np.asarray(freqs_cis, np.float32)
    cos = np.ascontiguousarray(fc[:, :, 0].T)   # [64, T]
    sin = np.ascontiguousarray(fc[:, :, 1].T)
    cosb = np.ascontiguousarray(np.tile(cos[IMAP], (1, B)), dtype=np.float32)
    sinb = np.ascontiguousarray(
        np.tile(sin[IMAP] * SSIGN[:, None], (1, B)), dtype=np.float32)
    # diagonal-block causal mask [key, (head, query)]: key <= query
    m = np.triu(np.ones((KB, QB), np.float32)).astype(BF16)
    mask = np.ascontiguousarray(np.tile(m, (1, HL)))
    return xT, cosb, sinb, mask


def _prep_core(d, wq_p, wk_p, wv_f, wo_f):
    qsl = slice(d * HL * HD, (d + 1) * HL * HD)
    ksl = slice(d * HD, (d + 1) * HD)
    wqT = np.ascontiguousarray(wq_p[qsl].T).astype(BF16)
    wkT = np.ascontiguousarray(wk_p[ksl].T).astype(BF16)
    wvT = np.ascontiguousarray(wv_f[ksl].T).astype(BF16)
    woT = np.ascontiguousarray(wo_f[:, qsl].T).astype(BF16)
    return wqT, wkT, wvT, woT


_NC_CACHE = []


def kernel(x, freqs_cis, wq, wk, wv, wo):
    from concourse import bass_utils

    if not _NC_CACHE:
        _NC_CACHE.append(_build())
    nc = _NC_CACHE[0]

    xT, cosb, sinb, mask = _prep_shared(x, freqs_cis)
    wq_p = np.asarray(wq, np.float32).reshape(H, HD, C)[:, PERM, :].reshape(H * HD, C)
    wk_p = np.asarray(wk, np.float32).reshape(KVH, HD, C)[:, PERM, :].reshape(KVH * HD, C)
    wv_f = np.asarray(wv, np.float32)
    wo_f = np.asarray(wo, np.float32)

    in_maps = []
    for d in range(NCORES):
        wqT, wkT, wvT, woT = _prep_core(d, wq_p, wk_p, wv_f, wo_f)
        in_maps.append({
            "xT": xT, "wqT": wqT, "wkT": wkT, "wvT": wvT, "woT": woT,
            "cosb": cosb, "sinb": sinb, "mask": mask,
        })
    res = bass_utils.run_bass_kernel_spmd(nc, in_maps, core_ids=list(range(NCORES)))
    acc = np.zeros((N, C), np.float32)
    for r in res.results:
        acc += np.asarray(r["out"], np.float32)
    return acc.reshape(B, T, C)


# revision 7
# speedup vs baseline: 1.1702x; 1.1261x over previous
"""Distributed GQA attention kernel for 8 TRN2 NeuronCores.

Strategy: tensor-parallel over heads, zero collectives.
Each core d holds 4 query heads + 1 kv head (GQA group d). It computes
q/k/v projections (transposed layouts), RoPE, causal attention, and a
partial o_proj (its heads' contribution to every output element). The
host sums the 8 partial outputs (the "unshard" step).

v2 layout decisions (all aimed at keeping the PE busy):
- Attention runs at 128-query granularity with all 4 local heads packed
  side by side, so score/AV/den matmuls stream 512 columns each.
- RoPE's rotate-half is a DVE stream_shuffle (32-lane group swap); the
  head-dim permutation is chosen so each pair's partner sits 16
  partitions away inside the same 32-partition quadrant.
- o_proj PSUM is double-buffered and its evacuations alternate between
  the Vector and Scalar engines.
- x is loaded 4 contraction-chunks per DMA; the o_proj partial output is
  written with one DMA per 128-token row block.
"""
import sys

sys.path.insert(0, '/opt/trn_rl_repo')

import numpy as np
import ml_dtypes

B, T, C = 2, 2048, 4096
H, KVH, HD = 32, 8, 128
NCORES = 8
N = B * T            # 4096 tokens (batches concatenated)
HL = H // NCORES     # 4 local q heads
TB = 256             # token block for projections
NTB = N // TB        # 16
QB = 128             # query block for attention
KB = 128             # key block
NCH = C // 128       # 32 contraction chunks
SCALE = float(1.0 / np.sqrt(HD))

# Head-dim permutation: pair i=(2i,2i+1) lives in quadrant i//16 at
# offsets i%16 (the "a" half) and 16+i%16 (the "b" half), so rotate-half
# becomes a 16<->16 swap inside each 32-partition stream_shuffle group.
PERM = np.empty(128, np.int64)
for _p in range(128):
    _qd, _r = _p // 32, _p % 32
    _i = _qd * 16 + (_r % 16)
    PERM[_p] = 2 * _i + (1 if _r >= 16 else 0)
IMAP = (np.arange(128) // 32) * 16 + (np.arange(128) % 32) % 16
SSIGN = np.where((np.arange(128) % 32) < 16, 1.0, -1.0).astype(np.float32)
SHUF = [(i + 16) % 32 for i in range(32)]

BF16 = ml_dtypes.bfloat16


def _build(dbg=False):
    import concourse.mybir as mybir
    import concourse.tile as tile
    from concourse import bacc

    dt = mybir.dt
    nc = bacc.Bacc("TRN2", target_bir_lowering=False, debug=False)

    xT_d = nc.declare_dram_parameter("xT", [C, N], dt.bfloat16, isOutput=False)
    wqT_d = nc.declare_dram_parameter("wqT", [C, HL * HD], dt.bfloat16, isOutput=False)
    wkT_d = nc.declare_dram_parameter("wkT", [C, HD], dt.bfloat16, isOutput=False)
    wvT_d = nc.declare_dram_parameter("wvT", [C, HD], dt.bfloat16, isOutput=False)
    woT_d = nc.declare_dram_parameter("woT", [HL * HD, C], dt.bfloat16, isOutput=False)
    cosb_d = nc.declare_dram_parameter("cosb", [128, N], dt.float32, isOutput=False)
    sinb_d = nc.declare_dram_parameter("sinb", [128, N], dt.float32, isOutput=False)
    mask_d = nc.declare_dram_parameter("mask", [128, HL * QB], dt.bfloat16, isOutput=False)
    out_d = nc.declare_dram_parameter("out", [N, C], dt.bfloat16, isOutput=True)

    with tile.TileContext(nc) as tc:
        with (
            tc.tile_pool(name="wts", bufs=1) as wts,
            tc.tile_pool(name="cache", bufs=1) as cache,
            tc.tile_pool(name="xin", bufs=14) as xin,
            tc.tile_pool(name="qk", bufs=2) as qkp,
            tc.tile_pool(name="rope", bufs=6) as ropep,
            tc.tile_pool(name="pt", bufs=6) as ptp,
            tc.tile_pool(name="att", bufs=2) as attp,
            tc.tile_pool(name="dn", bufs=2) as dnp,
            tc.tile_pool(name="oev", bufs=2) as oevp,
            tc.tile_pool(name="acc", bufs=2, space="PSUM") as accp,
            tc.tile_pool(name="sps", bufs=2, space="PSUM") as spsp,
            tc.tile_pool(name="avp", bufs=1, space="PSUM") as avpp,
            tc.tile_pool(name="dnp", bufs=1, space="PSUM") as dppp,
            tc.tile_pool(name="ops", bufs=2, space="PSUM") as opsp,
        ):
            # ---------------- resident weights / constants ----------------
            wq_s = wts.tile([128, NCH * HL * 128], dt.bfloat16)   # (c,h) -> col (c*HL+h)*128
            wk_s = wts.tile([128, NCH * 128], dt.bfloat16)
            wv_s = wts.tile([128, NCH * 128], dt.bfloat16)
            wo_s = wts.tile([128, HL * C], dt.bfloat16)           # (h,ct) -> col h*C+ct*512
            cos_s = wts.tile([128, N], dt.float32)
            sin_s = wts.tile([128, N], dt.float32)
            mask_s = wts.tile([128, HL * QB], dt.bfloat16)
            ones_s = wts.tile([128, 128], dt.bfloat16)

            nc.any.memset(ones_s[:, :], 1.0)
            # split big weight loads into pieces for DMA-queue parallelism
            wq_v = wq_s[:, :].rearrange("p (c m) -> p c m", c=NCH)
            wqT_v = wqT_d[:, :].rearrange("(c p) m -> p c m", p=128)
            wo_v = wo_s[:, :].rearrange("p (h m) -> p h m", h=HL)
            woT_v = woT_d[:, :].rearrange("(h p) m -> p h m", p=128)
            for i in range(8):
                cs = slice(i * (NCH // 8), (i + 1) * (NCH // 8))
                nc.sync.dma_start(wq_v[:, cs], wqT_v[:, cs])
            for i in range(HL):
                nc.sync.dma_start(wo_v[:, i], woT_v[:, i])
            wk_v = wk_s[:, :].rearrange("p (c m) -> p c m", c=NCH)
            wkT_v = wkT_d[:, :].rearrange("(c p) m -> p c m", p=128)
            wv_v = wv_s[:, :].rearrange("p (c m) -> p c m", c=NCH)
            wvT_v = wvT_d[:, :].rearrange("(c p) m -> p c m", p=128)
            for i in range(4):
                cs = slice(i * (NCH // 4), (i + 1) * (NCH // 4))
                nc.sync.dma_start(wk_v[:, cs], wkT_v[:, cs])
                nc.sync.dma_start(wv_v[:, cs], wvT_v[:, cs])
            for i in range(4):
                ns = slice(i * (N // 4), (i + 1) * (N // 4))
                nc.sync.dma_start(cos_s[:, ns], cosb_d[:, ns])
                nc.sync.dma_start(sin_s[:, ns], sinb_d[:, ns])
            nc.sync.dma_start(mask_s[:, :], mask_d[:, :])

            kcache = cache.tile([128, N], dt.bfloat16)   # [hd, tok]
            vcache = cache.tile([128, N], dt.bfloat16)   # [tok%128, blk*128+hd]

            def rope(dst, src, nsl):
                # dst = src*cos + shuffle16(src*sin'); all [128, TB]
                m1 = ropep.tile([128, TB], dt.float32, tag="m1")
                nc.vector.tensor_mul(m1[:, :], src, cos_s[:, nsl])
                u = ropep.tile([128, TB], dt.float32, tag="u")
                nc.vector.tensor_mul(u[:, :], src, sin_s[:, nsl])
                sw = ropep.tile([128, TB], dt.float32, tag="sw")
                nc.vector.stream_shuffle(sw[:, :], u[:, :], SHUF)
                nc.vector.tensor_add(dst, m1[:, :], sw[:, :])

            for tb in range(NTB):
                b = tb // 8
                nsl = slice(tb * TB, (tb + 1) * TB)
                # ================= A: projections for this token block ====
                # pass 1: q0|q1 and k|v0|v1 (2 banks); pass 2: q2|q3.
                xcs = []
                for ci in range(8):
                    xc = xin.tile([128, 4 * TB], dt.bfloat16, tag="xc")
                    xc_v = xc[:, :].rearrange("p (c m) -> p c m", c=4)
                    nc.sync.dma_start(
                        xc_v[:, :, :],
                        xT_d[:, nsl].rearrange("(c p) m -> p c m", p=128)[
                            :, ci * 4:(ci + 1) * 4])
                    xcs.append(xc_v)
                # q_sb layout: [hd, (h, qh, 128)] so the attention rhs for
                # query-half qh is the strided view [:, :, qh, :] (512 wide)
                q_sb = qkp.tile([128, HL * TB], dt.bfloat16, tag="qsb")
                q_qv = q_sb[:, :].rearrange("p (h q) -> p h q", h=HL)

                t0 = accp.tile([128, 512], dt.float32, tag="acc")  # q0|q1
                t2 = accp.tile([128, 512], dt.float32, tag="acc")  # k|v0|v1
                for c in range(NCH):
                    xc = xcs[c // 4][:, c % 4, :]
                    st = c == 0
                    sp = c == NCH - 1
                    for h in range(2):
                        nc.tensor.matmul(
                            t0[:, h * 256:(h + 1) * 256],
                            wq_s[:, (c * HL + h) * 128:(c * HL + h + 1) * 128],
                            xc, start=st and h == 0, stop=sp)
                    nc.tensor.matmul(
                        t2[:, 0:256],
                        wk_s[:, c * 128:(c + 1) * 128], xc,
                        start=st, stop=sp)
                    for ti in range(2):
                        nc.tensor.matmul(
                            t2[:, 256 + ti * 128:256 + (ti + 1) * 128],
                            xc[:, ti * 128:(ti + 1) * 128],
                            wv_s[:, c * 128:(c + 1) * 128], start=False, stop=sp)
                rope(q_qv[:, 0, :], t0[:, 0:256], nsl)
                rope(q_qv[:, 1, :], t0[:, 256:512], nsl)
                rope(kcache[:, nsl], t2[:, 0:256], nsl)
                for ti in range(2):
                    kbg = tb * 2 + ti
                    nc.vector.tensor_copy(
                        vcache[:, kbg * 128:(kbg + 1) * 128],
                        t2[:, 256 + ti * 128:256 + (ti + 1) * 128])
                t1 = accp.tile([128, 512], dt.float32, tag="acc")  # q2|q3
                for c in range(NCH):
                    xc = xcs[c // 4][:, c % 4, :]
                    sp = c == NCH - 1
                    for h in range(2):
                        nc.tensor.matmul(
                            t1[:, h * 256:(h + 1) * 256],
                            wq_s[:, (c * HL + h + 2) * 128:(c * HL + h + 3) * 128],
                            xc, start=c == 0 and h == 0, stop=sp)
                rope(q_qv[:, 2, :], t1[:, 0:256], nsl)
                rope(q_qv[:, 3, :], t1[:, 256:512], nsl)

                # ============ B+C: attention + o_proj per query block =====
                for qh in range(2):
                    qbl = (tb % 8) * 2 + qh        # in-batch 128-query block
                    nkb = qbl + 1
                    qrhs = q_qv[:, :, qh * 128:(qh + 1) * 128]
                    at4 = avpp.tile([128, 512], dt.float32, tag="at4")
                    den = dppp.tile([128, 512], dt.float32, tag="den")
                    for kbl in range(nkb):
                        kbg = b * 16 + kbl
                        ksl = slice(kbg * 128, (kbg + 1) * 128)
                        sT = spsp.tile([128, 512], dt.float32, tag="sT")
                        nc.tensor.matmul(sT[:, :], kcache[:, ksl], qrhs,
                                         start=True, stop=True)
                        pT = ptp.tile([128, 512], dt.bfloat16, tag="pT")
                        nc.scalar.activation(pT[:, :], sT[:, :],
                                             mybir.ActivationFunctionType.Exp,
                                             scale=SCALE)
                        if kbl == nkb - 1:
                            nc.vector.tensor_mul(pT[:, :], pT[:, :], mask_s[:, :])
                        st = kbl == 0
                        sp = kbl == nkb - 1
                        nc.tensor.matmul(at4[:, :], vcache[:, ksl], pT[:, :],
                                         start=st, stop=sp)
                        nc.tensor.matmul(den[:, :], ones_s[:, :], pT[:, :],
                                         start=st, stop=sp)
                    denb = dnp.tile([128, 512], dt.float32, tag="denb")
                    nc.vector.reciprocal_approx_fast(denb[:, :], den[:, :])
                    attn = attp.tile([128, 512], dt.bfloat16, tag="attn")
                    nc.vector.tensor_mul(attn[:, :], at4[:, :], denb[:, :])

                    # -------- C: partial o_proj for these 128 tokens ------
                    r0 = tb * TB + qh * 128
                    oev = oevp.tile([128, C], dt.bfloat16, tag="oev")
                    for ct in range(C // 512):
                        ops = opsp.tile([128, 512], dt.float32, tag="ops")
                        for h in range(HL):
                            nc.tensor.matmul(
                                ops[:, :],
                                attn[:, h * 128:(h + 1) * 128],
                                wo_s[:, h * C + ct * 512:h * C + (ct + 1) * 512],
                                start=(h == 0), stop=(h == HL - 1))
                        osl = slice(ct * 512, (ct + 1) * 512)
                        if ct % 2 == 0:
                            nc.vector.tensor_copy(oev[:, osl], ops[:, :])
                        else:
                            nc.scalar.activation(
                                oev[:, osl], ops[:, :],
                                mybir.ActivationFunctionType.Copy)
                    nc.gpsimd.dma_start(out_d[r0:r0 + 128, :], oev[:, :])
    nc.finalize()
    return nc


def _prep_shared(x, freqs_cis):
    xf = np.asarray(x, np.float32).reshape(N, C)
    xT = np.ascontiguousarray(xf.T).astype(BF16)
    fc = np.asarray(freqs_cis, np.float32)
    cos = np.ascontiguousarray(fc[:, :, 0].T)   # [64, T]
    sin = np.ascontiguousarray(fc[:, :, 1].T)
    cosb = np.ascontiguousarray(np.tile(cos[IMAP], (1, B)), dtype=np.float32)
    sinb = np.ascontiguousarray(
        np.tile(sin[IMAP] * SSIGN[:, None], (1, B)), dtype=np.float32)
    # diagonal-block causal mask [key, (head, query)]: key <= query
    m = np.triu(np.ones((KB, QB), np.float32)).astype(BF16)
    mask = np.ascontiguousarray(np.tile(m, (1, HL)))
    return xT, cosb, sinb, mask


def _prep_core(d, wq_p, wk_p, wv_f, wo_f):
    qsl = slice(d * HL * HD, (d + 1) * HL * HD)
    ksl = slice(d * HD, (d + 1) * HD)
    wqT = np.ascontiguousarray(wq_p[qsl].T).astype(BF16)
    wkT = np.ascontiguousarray(wk_p[ksl].T).astype(BF16)
    wvT = np.ascontiguousarray(wv_f[ksl].T).astype(BF16)
    woT = np.ascontiguousarray(wo_f[:, qsl].T).astype(BF16)
    return wqT, wkT, wvT, woT


_NC_CACHE = []


def kernel(x, freqs_cis, wq, wk, wv, wo):
    from concourse import bass_utils

    if not _NC_CACHE:
        _NC_CACHE.append(_build())
    nc = _NC_CACHE[0]

    xT, cosb, sinb, mask = _prep_shared(x, freqs_cis)
    wq_p = np.asarray(wq, np.float32).reshape(H, HD, C)[:, PERM, :].reshape(H * HD, C)
    wk_p = np.asarray(wk, np.float32).reshape(KVH, HD, C)[:, PERM, :].reshape(KVH * HD, C)
    wv_f = np.asarray(wv, np.float32)
    wo_f = np.asarray(wo, np.float32)

    in_maps = []
    for d in range(NCORES):
        wqT, wkT, wvT, woT = _prep_core(d, wq_p, wk_p, wv_f, wo_f)
        in_maps.append({
            "xT": xT, "wqT": wqT, "wkT": wkT, "wvT": wvT, "woT": woT,
            "cosb": cosb, "sinb": sinb, "mask": mask,
        })
    res = bass_utils.run_bass_kernel_spmd(nc, in_maps, core_ids=list(range(NCORES)))
    acc = np.zeros((N, C), np.float32)
    for r in res.results:
        acc += np.asarray(r["out"], np.float32)
    return acc.reshape(B, T, C)
